# revision 64
# baseline (speedup 1.0000x reference)
"""Multi-head attention (b=16, n=512, d=768, h=12) on 8 trn2 NeuronCores.

Strategy: pure data-parallel over batch (2 batches per core), no collectives.
Host pre-transposes/casts the per-core x slice to xT bf16 [768, 1024] and
casts weights to bf16; all matmuls run bf16 with fp32 PSUM accumulation.

Per-core dataflow (P = 128 partitions):
  qkT[m]  = Wqkv[:, m-tile]^T @ xT          -> [outfeat, tok] (bf16; bias
            added on the PSUM->SBUF copy by DVE tensor_scalar)
  vaug    = x @ Wv stored per head as [v_h | ones1]  (65 cols per head)
  scoresT = k_h @ q_h^T  (2 heads row-packed, K=64 each, into one
            [128,1024] PSUM tile; one merged exp -> attnT bf16)
  ctx     = attnT_chunk^T @ vaug_h: token-major [q, 64v | den] slots —
            the ones column gives the softmax denominator per (q, head)
  cn      = ctx * (1/den)  (DVE tensor_scalar, per-partition scalar)
  ctxT    = XBAR DMA transpose of cn  -> feature-major [2*dh, q]
  out     = ctxT^T @ Wo + bo            (natural [tok, feat], DMA out)

Emission is software-pipelined: qk-projection units and the batch-0
out-projections are interleaved into the attention pairs as PE filler so
the Tensor engine never waits on the (slower) exp activations.
"""

import numpy as np
import ml_dtypes

import concourse.bass as bass
import concourse.mybir as mybir
import concourse.tile as tile
from concourse import bacc
from concourse.bass_utils import run_bass_kernel_spmd

# Problem constants (hardcoded per contest contract).
B = 16          # global batch
N = 512         # sequence length
D = 768         # embed dim
H = 12          # heads
DH = 64         # head dim
NCORES = 8
BPC = B // NCORES          # batches per core = 2
TOK = BPC * N              # tokens per core = 1024
P = 128
KC = D // P                # 6 contraction chunks
NQK = 2 * D // P           # 12 q+k m-tiles
TT = TOK // P              # 8 token tiles
HPAIRS = H // 2            # 6 head pairs
VW = DH + 1                # 65 cols per head in vaug (v | ones)

F32 = mybir.dt.float32
BF16 = mybir.dt.bfloat16
BF16_NP = ml_dtypes.bfloat16

# Module-level knobs (test.py pokes these; harness uses defaults).
TRACE = False
LAST_EXEC_NS = None
LAST_RESULTS = None

_CACHED_NC = None


def _build_nc():
    # Bacc (not raw Bass): its compile() splits sync-waits to satisfy the
    # TRN2 1-wait-per-instruction codegen constraint.
    nc = bacc.Bacc(None, target_bir_lowering=False)
    xt = nc.declare_dram_parameter("xt", [D, TOK], BF16, isOutput=False)
    wqkv = nc.declare_dram_parameter("wqkv", [D, 3 * D], BF16, isOutput=False)
    bqkv = nc.declare_dram_parameter("bqkv", [3 * D], F32, isOutput=False)
    wo = nc.declare_dram_parameter("wo", [D, D], BF16, isOutput=False)
    bo = nc.declare_dram_parameter("bo", [D], F32, isOutput=False)
    # bf16 output: halves the (serialized) store traffic in the drain tail;
    # the host upcasts. Costs ~2e-3 extra rel err against the 2e-2 gate.
    out = nc.declare_dram_parameter("out", [TOK, D], BF16, isOutput=True)

    with tile.TileContext(nc) as tc:
        _body(tc, xt, wqkv, bqkv, wo, bo, out)
    nc.compile()
    return nc


def _body(tc, xt, wqkv, bqkv, wo, bo, out):
    nc = tc.nc
    AOP = mybir.AluOpType
    ACTF = mybir.ActivationFunctionType

    with (
        tc.tile_pool(name="consts", bufs=1) as consts,
        tc.tile_pool(name="work", bufs=2) as work,
        tc.tile_pool(name="psum", bufs=2, space="PSUM") as psum,
    ):
        # ---- persistent SBUF tensors -------------------------------------
        xt_sb = [consts.tile([P, TOK], BF16, tag=f"xt{k}", name=f"xt{k}") for k in range(KC)]
        wqkv_sb = [consts.tile([P, 3 * D], BF16, tag=f"wqkv{k}", name=f"wqkv{k}") for k in range(KC)]
        wo_sb = [consts.tile([P, D], BF16, tag=f"wo{k}", name=f"wo{k}") for k in range(KC)]
        bqk_sb = consts.tile([P, NQK], F32, tag="bqk")
        bv_sb = consts.tile([P, D], F32, tag="bv")
        bo_sb = consts.tile([P, D], F32, tag="bo")
        qkT = [consts.tile([P, TOK], BF16, tag=f"qkT{m}", name=f"qkT{m}") for m in range(NQK)]
        # vaug[t]: per head h, cols 65h..65h+64 = v values, col 65h+64 = 1.0
        vaug = [consts.tile([P, H * VW], BF16, tag=f"vaug{t}", name=f"vaug{t}") for t in range(TT)]
        ctxT = [consts.tile([P, N], BF16, tag=f"ctxT{i}", name=f"ctxT{i}") for i in range(BPC * HPAIRS)]

        # ---- loads: xt on the SP ring, wqkv v-cols in parallel on the ACT
        # ring (idle this early), so the v_proj(0..3) ramp work unblocks
        # chunk-by-chunk; the 2x larger q/k columns stream in behind and are
        # consumed by the later qk_proj phase.
        nc.sync.dma_start(out=xt_sb[0][:, 0:P], in_=xt[0:P, 0:P])
        nc.scalar.dma_start(out=wqkv_sb[0][:, 2 * D:2 * D + 512],
                            in_=wqkv[0:P, 2 * D:2 * D + 512])
        nc.sync.dma_start(out=xt_sb[0][:, P:256], in_=xt[0:P, P:256])
        nc.scalar.dma_start(out=wqkv_sb[0][:, 2 * D + 512:3 * D],
                            in_=wqkv[0:P, 2 * D + 512:3 * D])
        # first two token tiles of xt (v0/v1 inputs), then v-cols per chunk;
        # the last two v-col chunks ride the SWDGE (gpsimd) ring so the ACT
        # ring finishes sooner.
        for k in range(1, KC):
            nc.sync.dma_start(out=xt_sb[k][:, 0:256], in_=xt[k * P:(k + 1) * P, 0:256])
            if k < 4:
                nc.scalar.dma_start(out=wqkv_sb[k][:, 2 * D:3 * D],
                                    in_=wqkv[k * P:(k + 1) * P, 2 * D:3 * D])
        # Remaining SP-ring loads ordered just ahead of demand: xt tiles for
        # v2/v3, then q/k m-pairs (0,1)+(6,7) for the first qk units, xt for
        # v4/v5, then the remaining m-pairs in first-use order.
        def _xt_chunk(c0, c1):
            for k in range(KC):
                nc.sync.dma_start(out=xt_sb[k][:, c0:c1],
                                  in_=xt[k * P:(k + 1) * P, c0:c1])

        def _mp_chunk(mp, eng=None):
            ring = nc.scalar if eng == "scalar" else nc.sync
            for k in range(KC):
                ring.dma_start(
                    out=wqkv_sb[k][:, mp * 256:(mp + 1) * 256],
                    in_=wqkv[k * P:(k + 1) * P, mp * 256:(mp + 1) * 256])

        # Early SWDGE loads: q/k bias (per-partition layout bqk_sb[p, m] =
        # bqkv[m*128 + p]), broadcast v bias, and v-cols k=4,5.
        nc.gpsimd.dma_start(
            out=bqk_sb, in_=bqkv[0:2 * D].rearrange("(m p) -> p m", p=P))
        bqkv_ap = bqkv[:]
        nc.gpsimd.dma_start(
            out=bv_sb,
            in_=bass.AP(tensor=bqkv_ap.tensor, offset=2 * D, ap=[[0, P], [1, D]]))
        for k in (4, 5):
            nc.gpsimd.dma_start(out=wqkv_sb[k][:, 2 * D:3 * D],
                                in_=wqkv[k * P:(k + 1) * P, 2 * D:3 * D])
        # the first-needed q/k m-pairs ride the ACT ring (free after the
        # v-cols land); the rest follow the xt remainder on SP.
        _mp_chunk(0, eng="scalar")
        _mp_chunk(3, eng="scalar")
        _xt_chunk(256, TOK)
        for mp in (1, 4, 2, 5):
            _mp_chunk(mp)
        # bo and Wo ride the SWDGE ring last: first needed by the out
        # projections near t~80us, so they must not delay the xt/wqkv
        # stream on the (globally serialized) DMA engines.
        bo_ap = bo[:]
        nc.gpsimd.dma_start(
            out=bo_sb,
            in_=bass.AP(tensor=bo_ap.tensor, offset=0, ap=[[0, P], [1, D]]))
        for k in range(KC):
            nc.gpsimd.dma_start(out=wo_sb[k], in_=wo[k * P:(k + 1) * P, :])
        # ones columns of v_aug (persistent; written once). On the vector
        # engine: the v_proj STT (also DVE) then needs no cross-engine wait.
        for t in range(TT):
            ones_view = vaug[t].rearrange("p (h x) -> p h x", x=VW)[:, :, DH:VW]
            nc.vector.memset(ones_view, 1.0)
        # Pre-observe the bias DMAs on the engines that consume them, so the
        # hot-loop ops carry only their PE wait.
        scratch = consts.tile([1, 4], F32, tag="scratch")
        nc.vector.tensor_copy(out=scratch[0:1, 0:1], in_=bv_sb[0:1, 0:1])
        nc.vector.tensor_copy(out=scratch[0:1, 1:2], in_=bo_sb[0:1, 0:1])
        nc.vector.tensor_copy(out=scratch[0:1, 2:3], in_=bqk_sb[0:1, 0:1])
        # dummy exp: pulls the ACT function-table load into the idle S1
        # phase instead of delaying the first real exp
        nc.scalar.activation(out=scratch[0:1, 3:4], in_=scratch[0:1, 0:1],
                             func=ACTF.Exp, scale=1.0)

        # ---- v-projection: one [128,1024] psum (768 used), 2 mm chains ---
        # Early v tiles alternate between the "sc" and (still idle) "ctx"
        # psum tags so the S1 phase isn't throttled by sc's 2-deep rotation.
        def v_proj(t, tag="sc"):
            ps = psum.tile([P, 1024], F32, tag=tag,
                           bufs=1 if tag == "ctx" else None)
            for k in range(KC):
                lhsT = xt_sb[k][:, t * P:(t + 1) * P]
                nc.tensor.matmul(ps[:, 0:512], lhsT,
                                 wqkv_sb[k][:, 2 * D:2 * D + 512],
                                 start=(k == 0), stop=(k == KC - 1))
                nc.tensor.matmul(ps[:, 512:768], lhsT,
                                 wqkv_sb[k][:, 2 * D + 512:3 * D],
                                 start=(k == 0), stop=(k == KC - 1))
            vview = vaug[t].rearrange("p (h x) -> p h x", x=VW)
            bview = bv_sb.rearrange("p (h x) -> p h x", x=DH)
            nc.vector.scalar_tensor_tensor(
                out=vview[:, 0:8, 0:DH],
                in0=ps[:, 0:512].rearrange("p (h x) -> p h x", x=DH),
                scalar=1.0, in1=bview[:, 0:8, :],
                op0=AOP.mult, op1=AOP.add)
            nc.vector.scalar_tensor_tensor(
                out=vview[:, 8:12, 0:DH],
                in0=ps[:, 512:768].rearrange("p (h x) -> p h x", x=DH),
                scalar=1.0, in1=bview[:, 8:12, :],
                op0=AOP.mult, op1=AOP.add)

        # ---- q/k projection unit: one m-tile, one token half -------------
        def qk_unit(m, tch, eng="vector"):
            ps = psum.tile([P, 512], F32, tag="qk")
            for k in range(KC):
                nc.tensor.matmul(
                    ps,
                    wqkv_sb[k][:, m * P:(m + 1) * P],
                    xt_sb[k][:, tch * 512:(tch + 1) * 512],
                    start=(k == 0), stop=(k == KC - 1))
            # PSUM->SBUF copy with per-partition bias add. Early units drain
            # on ACT (no exps yet); later ones on DVE (ACT is exp-saturated).
            if eng == "scalar":
                nc.scalar.activation(
                    out=qkT[m][:, tch * 512:(tch + 1) * 512], in_=ps,
                    func=ACTF.Identity, bias=bqk_sb[:, m:m + 1], scale=1.0)
            else:
                nc.vector.tensor_scalar(
                    out=qkT[m][:, tch * 512:(tch + 1) * 512], in0=ps,
                    scalar1=bqk_sb[:, m:m + 1], scalar2=None, op0=AOP.add)

        # ---- attention: scores+exp per (b, hp, kc); ctx per (b, hp) ------
        def scores(b, hp, kc, attn_tiles):
            ktile, qtile = qkT[HPAIRS + hp], qkT[hp]
            ps = psum.tile([P, 1024], F32, tag="sc")
            for hh in range(2):
                pr = slice(64 * hh, 64 * hh + 64)
                nc.tensor.matmul(
                    ps[:, hh * 512:(hh + 1) * 512],
                    ktile[pr, b * N + kc * P: b * N + (kc + 1) * P],
                    qtile[pr, b * N:(b + 1) * N],
                    start=True, stop=True)
            at = work.tile([P, 1024], BF16, tag="attn", bufs=14)
            nc.scalar.activation(out=at, in_=ps, func=ACTF.Exp,
                                 scale=1.0 / np.sqrt(DH))
            attn_tiles[kc] = at

        def ctx_group(b, hp, attn_tiles):
            # token-major ctx accumulation: slot s = (qc, hh) occupies cols
            # [128s, 128s+65): 64 v dims + softmax denominator (ones col).
            # PSUM zero-region semantics: start=True zeroes the full 2KB bank,
            # so exactly one start (first matmul into the bank) and one stop
            # (last matmul out of it); the other slots accumulate onto the
            # zeroed region.
            cps = psum.tile([P, 1024], F32, tag="ctx", bufs=1)
            for kc in range(4):
                at = attn_tiles[kc]
                for qc in range(4):
                    for hh in range(2):
                        s = qc * 2 + hh
                        h = 2 * hp + hh
                        nc.tensor.matmul(
                            cps[:, s * P: s * P + VW],
                            at[:, hh * 512 + qc * P: hh * 512 + (qc + 1) * P],
                            vaug[b * 4 + kc][:, h * VW:(h + 1) * VW],
                            start=(kc == 0 and s % 4 == 0),
                            stop=(kc == 3 and s % 4 == 3))
            # drain: reciprocal of the 8 denominators, normalize to bf16,
            # then XBAR-transpose each [128q, 128dh2] block into ctxT.
            bc = work.tile([P, 8], F32, tag="bc", bufs=6)
            den = cps.rearrange("p (s x) -> p s x", x=P)[:, :, DH:DH + 1]
            nc.vector.reciprocal(out=bc, in_=den)
            cns = [work.tile([P, P], BF16, tag="cn", bufs=12, name=f"cn{qc}")
                   for qc in range(4)]
            for s in range(8):
                qc, hh = s // 2, s % 2
                nc.vector.tensor_scalar(
                    out=cns[qc][:, hh * DH:(hh + 1) * DH],
                    in0=cps[:, s * P: s * P + DH],
                    scalar1=bc[:, s:s + 1], scalar2=None, op0=AOP.mult)
                if hh == 1:
                    nc.scalar.dma_start_transpose(
                        out=ctxT[b * HPAIRS + hp][:, qc * P:(qc + 1) * P],
                        in_=cns[qc])

        def out_proj_mm(b, tt_in_b):
            ps = psum.tile([P, 1024], F32, tag="sc")
            for hp in range(HPAIRS):
                lhsT = ctxT[b * HPAIRS + hp][:, tt_in_b * P:(tt_in_b + 1) * P]
                nc.tensor.matmul(ps[:, 0:512], lhsT, wo_sb[hp][:, 0:512],
                                 start=(hp == 0), stop=(hp == HPAIRS - 1))
                nc.tensor.matmul(ps[:, 512:768], lhsT, wo_sb[hp][:, 512:D],
                                 start=(hp == 0), stop=(hp == HPAIRS - 1))
            return ps

        def out_drain(b, tt_in_b, ps, chunks=1):
            t = b * 4 + tt_in_b
            o = work.tile([P, D], BF16, tag="out", bufs=6)
            edges = [round(D * i / chunks) for i in range(chunks + 1)]
            for c0, c1 in zip(edges, edges[1:]):
                nc.vector.scalar_tensor_tensor(
                    out=o[:, c0:c1], in0=ps[:, c0:c1], scalar=1.0,
                    in1=bo_sb[:, c0:c1], op0=AOP.mult, op1=AOP.add)
                nc.sync.dma_start(out=out[t * P:(t + 1) * P, c0:c1],
                                  in_=o[:, c0:c1])

        def out_proj(b, tt_in_b, chunks=1):
            out_drain(b, tt_in_b, out_proj_mm(b, tt_in_b), chunks)

        def out_proj_final(b, tt_in_b):
            # Last tile: bank-sequential matmul chains in SEPARATE psum
            # tiles (PSUM dep tracking is tile-granular) so chunk 0's
            # bias-add STT and store issue while chunk 1's matmuls still
            # run. Minimizes the PE-end -> last-store tail.
            t = b * 4 + tt_in_b
            o = work.tile([P, D], BF16, tag="out", bufs=6)
            for c0, c1 in ((0, 512), (512, D)):
                ps = psum.tile([P, 1024], F32, tag="sc", name=f"psf{c0}")
                for hp in range(HPAIRS):
                    lhsT = ctxT[b * HPAIRS + hp][:, tt_in_b * P:(tt_in_b + 1) * P]
                    nc.tensor.matmul(ps[:, 0:c1 - c0], lhsT, wo_sb[hp][:, c0:c1],
                                     start=(hp == 0), stop=(hp == HPAIRS - 1))
                nc.vector.scalar_tensor_tensor(
                    out=o[:, c0:c1], in0=ps[:, 0:c1 - c0], scalar=1.0,
                    in1=bo_sb[:, c0:c1], op0=AOP.mult, op1=AOP.add)
                nc.sync.dma_start(out=out[t * P:(t + 1) * P, c0:c1],
                                  in_=o[:, c0:c1])

        # ---- emission schedule (software pipeline) -----------------------
        # S1: batch-0 v tiles + the first qk units, overlapped with loads.
        for t in range(4):
            v_proj(t, tag="sc" if t % 2 == 0 else "ctx")
        qk_unit(0, 0)
        qk_unit(HPAIRS, 0)

        # S2: per head-pair, both batches' attention with qk/v/out units as
        # PE filler between the score matmuls (keeps PE busy while ACT exps).
        # Each iteration has 8 fill slots; the qk units for the NEXT pair go
        # in the batch-1 half so both halves stay PE-bound.
        for hp in range(HPAIRS):
            at0, at1 = {}, {}
            last = hp == HPAIRS - 1
            # filler slots F1..F8, None = skip
            F = [None] * 8
            F[0] = lambda hp=hp: qk_unit(hp, 1)
            F[1] = lambda hp=hp: qk_unit(HPAIRS + hp, 1)
            if hp == 0:
                F[0] = lambda: v_proj(4)
                F[1] = lambda: v_proj(5)
                F[2] = lambda: qk_unit(0, 1)
                F[3] = lambda: qk_unit(HPAIRS, 1)
                F[6] = lambda: v_proj(6)
                F[7] = lambda: v_proj(7)
            if not last:
                F[4] = lambda hp=hp: qk_unit(hp + 1, 0)
                F[5] = lambda hp=hp: qk_unit(HPAIRS + hp + 1, 0)

            def fill(i):
                if F[i] is not None:
                    F[i]()

            scores(0, hp, 0, at0)
            scores(0, hp, 1, at0)
            fill(0)
            scores(0, hp, 2, at0)
            fill(1)
            scores(0, hp, 3, at0)
            fill(2)
            fill(3)
            ctx_group(0, hp, at0)
            scores(1, hp, 0, at1)
            fill(4)
            scores(1, hp, 1, at1)
            fill(5)
            scores(1, hp, 2, at1)
            fill(6)
            # batch-0 ctxT completes in this iteration; out_proj(0,0)'s
            # matmuls are PE filler for the final pair's exps, but its
            # stores are deferred until after ctx_group(1,5)'s transposes
            # so the XBARs sit early in the HWDGE chain.
            ps00 = out_proj_mm(0, 0) if last else None
            scores(1, hp, 3, at1)
            fill(7)
            ctx_group(1, hp, at1)
            if last:
                out_drain(0, 0, ps00)

        # S3: remaining out projections; final tile drains in small chunks
        # so the PE->DVE->DMA tail is short.
        for tt_in_b in range(1, 4):
            out_proj(0, tt_in_b)
        for tt_in_b in range(3):
            out_proj(1, tt_in_b)
        out_proj_final(1, 3)


def _get_nc():
    global _CACHED_NC
    if _CACHED_NC is None:
        _CACHED_NC = _build_nc()
    return _CACHED_NC


def kernel(x, Wqkv, bqkv, Wo, bo):
    global LAST_EXEC_NS, LAST_RESULTS
    x = np.asarray(x, dtype=np.float32)
    wqkv_bf = np.asarray(Wqkv, dtype=np.float32).astype(BF16_NP)
    wo_bf = np.asarray(Wo, dtype=np.float32).astype(BF16_NP)
    bqkv_f = np.ascontiguousarray(np.asarray(bqkv, dtype=np.float32))
    bo_f = np.ascontiguousarray(np.asarray(bo, dtype=np.float32))

    in_maps = []
    for c in range(NCORES):
        xc = x[c * BPC:(c + 1) * BPC].reshape(TOK, D).T  # [768, 1024]
        in_maps.append({
            "xt": np.ascontiguousarray(xc).astype(BF16_NP),
            "wqkv": wqkv_bf,
            "bqkv": bqkv_f,
            "wo": wo_bf,
            "bo": bo_f,
        })

    nc = _get_nc()
    res = run_bass_kernel_spmd(nc, in_maps, list(range(NCORES)), trace=TRACE)
    LAST_EXEC_NS = res.exec_time_ns
    LAST_RESULTS = res
    outs = [np.asarray(res.results[c]["out"]).astype(np.float32)
            for c in range(NCORES)]
    return np.concatenate(outs, axis=0).reshape(B, N, D)


# revision 69
# speedup vs baseline: 1.0028x; 1.0028x over previous
"""Multi-head attention (b=16, n=512, d=768, h=12) on 8 trn2 NeuronCores.

Strategy: pure data-parallel over batch (2 batches per core), no collectives.
Host pre-transposes/casts the per-core x slice to xT bf16 [768, 1024] and
casts weights to bf16; all matmuls run bf16 with fp32 PSUM accumulation.

Per-core dataflow (P = 128 partitions):
  qkT[m]  = Wqkv[:, m-tile]^T @ xT          -> [outfeat, tok] (bf16; bias
            added on the PSUM->SBUF copy by DVE tensor_scalar)
  vaug    = x @ Wv stored per head as [v_h | ones1]  (65 cols per head)
  scoresT = k_h @ q_h^T  (2 heads row-packed, K=64 each, into one
            [128,1024] PSUM tile; one merged exp -> attnT bf16)
  ctx     = attnT_chunk^T @ vaug_h: token-major [q, 64v | den] slots —
            the ones column gives the softmax denominator per (q, head)
  cn      = ctx * (1/den)  (DVE tensor_scalar, per-partition scalar)
  ctxT    = XBAR DMA transpose of cn  -> feature-major [2*dh, q]
  out     = ctxT^T @ Wo + bo            (natural [tok, feat], DMA out)

Emission is software-pipelined: qk-projection units and the batch-0
out-projections are interleaved into the attention pairs as PE filler so
the Tensor engine never waits on the (slower) exp activations.
"""

import numpy as np
import ml_dtypes

import concourse.bass as bass
import concourse.mybir as mybir
import concourse.tile as tile
from concourse import bacc
from concourse.bass_utils import run_bass_kernel_spmd

# Problem constants (hardcoded per contest contract).
B = 16          # global batch
N = 512         # sequence length
D = 768         # embed dim
H = 12          # heads
DH = 64         # head dim
NCORES = 8
BPC = B // NCORES          # batches per core = 2
TOK = BPC * N              # tokens per core = 1024
P = 128
KC = D // P                # 6 contraction chunks
NQK = 2 * D // P           # 12 q+k m-tiles
TT = TOK // P              # 8 token tiles
HPAIRS = H // 2            # 6 head pairs
VW = DH + 1                # 65 cols per head in vaug (v | ones)

F32 = mybir.dt.float32
BF16 = mybir.dt.bfloat16
BF16_NP = ml_dtypes.bfloat16

# Module-level knobs (test.py pokes these; harness uses defaults).
TRACE = False
LAST_EXEC_NS = None
LAST_RESULTS = None

_CACHED_NC = None


def _build_nc():
    # Bacc (not raw Bass): its compile() splits sync-waits to satisfy the
    # TRN2 1-wait-per-instruction codegen constraint.
    nc = bacc.Bacc(None, target_bir_lowering=False)
    xt = nc.declare_dram_parameter("xt", [D, TOK], BF16, isOutput=False)
    wqkv = nc.declare_dram_parameter("wqkv", [D, 3 * D], BF16, isOutput=False)
    bqkv = nc.declare_dram_parameter("bqkv", [3 * D], F32, isOutput=False)
    wo = nc.declare_dram_parameter("wo", [D, D], BF16, isOutput=False)
    bo = nc.declare_dram_parameter("bo", [D], F32, isOutput=False)
    # bf16 output: halves the (serialized) store traffic in the drain tail;
    # the host upcasts. Costs ~2e-3 extra rel err against the 2e-2 gate.
    out = nc.declare_dram_parameter("out", [TOK, D], BF16, isOutput=True)

    with tile.TileContext(nc) as tc:
        _body(tc, xt, wqkv, bqkv, wo, bo, out)
    nc.compile()
    return nc


def _body(tc, xt, wqkv, bqkv, wo, bo, out):
    nc = tc.nc
    AOP = mybir.AluOpType
    ACTF = mybir.ActivationFunctionType

    with (
        tc.tile_pool(name="consts", bufs=1) as consts,
        tc.tile_pool(name="work", bufs=2) as work,
        tc.tile_pool(name="psum", bufs=2, space="PSUM") as psum,
    ):
        # ---- persistent SBUF tensors -------------------------------------
        xt_sb = [consts.tile([P, TOK], BF16, tag=f"xt{k}", name=f"xt{k}") for k in range(KC)]
        wqkv_sb = [consts.tile([P, 3 * D], BF16, tag=f"wqkv{k}", name=f"wqkv{k}") for k in range(KC)]
        wo_sb = [consts.tile([P, D], BF16, tag=f"wo{k}", name=f"wo{k}") for k in range(KC)]
        bqk_sb = consts.tile([P, NQK], F32, tag="bqk")
        bv_sb = consts.tile([P, D], F32, tag="bv")
        bo_sb = consts.tile([P, D], F32, tag="bo")
        qkT = [consts.tile([P, TOK], BF16, tag=f"qkT{m}", name=f"qkT{m}") for m in range(NQK)]
        # vaug[t]: per head h, cols 65h..65h+64 = v values, col 65h+64 = 1.0
        vaug = [consts.tile([P, H * VW], BF16, tag=f"vaug{t}", name=f"vaug{t}") for t in range(TT)]
        ctxT = [consts.tile([P, N], BF16, tag=f"ctxT{i}", name=f"ctxT{i}") for i in range(BPC * HPAIRS)]
        # ones row + bf16 bo row: K=1 bias matmul for the final out chunk,
        # so its drain is a pure ACT copy (off the busy DVE/SP tail path)
        ones_row = consts.tile([1, P], BF16, tag="ones_row")
        bo_row = consts.tile([1, D], BF16, tag="bo_row")

        # ---- loads: xt on the SP ring, wqkv v-cols in parallel on the ACT
        # ring (idle this early), so the v_proj(0..3) ramp work unblocks
        # chunk-by-chunk; the 2x larger q/k columns stream in behind and are
        # consumed by the later qk_proj phase.
        nc.sync.dma_start(out=xt_sb[0][:, 0:P], in_=xt[0:P, 0:P])
        nc.scalar.dma_start(out=wqkv_sb[0][:, 2 * D:2 * D + 512],
                            in_=wqkv[0:P, 2 * D:2 * D + 512])
        nc.sync.dma_start(out=xt_sb[0][:, P:256], in_=xt[0:P, P:256])
        nc.scalar.dma_start(out=wqkv_sb[0][:, 2 * D + 512:3 * D],
                            in_=wqkv[0:P, 2 * D + 512:3 * D])
        # first two token tiles of xt (v0/v1 inputs), then v-cols per chunk;
        # the last two v-col chunks ride the SWDGE (gpsimd) ring so the ACT
        # ring finishes sooner.
        for k in range(1, KC):
            nc.sync.dma_start(out=xt_sb[k][:, 0:256], in_=xt[k * P:(k + 1) * P, 0:256])
            if k < 4:
                nc.scalar.dma_start(out=wqkv_sb[k][:, 2 * D:3 * D],
                                    in_=wqkv[k * P:(k + 1) * P, 2 * D:3 * D])
        # Remaining SP-ring loads ordered just ahead of demand: xt tiles for
        # v2/v3, then q/k m-pairs (0,1)+(6,7) for the first qk units, xt for
        # v4/v5, then the remaining m-pairs in first-use order.
        def _xt_chunk(c0, c1):
            for k in range(KC):
                nc.sync.dma_start(out=xt_sb[k][:, c0:c1],
                                  in_=xt[k * P:(k + 1) * P, c0:c1])

        def _mp_chunk(mp, eng=None):
            ring = nc.scalar if eng == "scalar" else nc.sync
            for k in range(KC):
                ring.dma_start(
                    out=wqkv_sb[k][:, mp * 256:(mp + 1) * 256],
                    in_=wqkv[k * P:(k + 1) * P, mp * 256:(mp + 1) * 256])

        # Early SWDGE loads: q/k bias (per-partition layout bqk_sb[p, m] =
        # bqkv[m*128 + p]), broadcast v bias, and v-cols k=4,5.
        nc.gpsimd.dma_start(
            out=bqk_sb, in_=bqkv[0:2 * D].rearrange("(m p) -> p m", p=P))
        bqkv_ap = bqkv[:]
        nc.gpsimd.dma_start(
            out=bv_sb,
            in_=bass.AP(tensor=bqkv_ap.tensor, offset=2 * D, ap=[[0, P], [1, D]]))
        for k in (4, 5):
            nc.gpsimd.dma_start(out=wqkv_sb[k][:, 2 * D:3 * D],
                                in_=wqkv[k * P:(k + 1) * P, 2 * D:3 * D])
        # the first-needed q/k m-pairs ride the ACT ring (free after the
        # v-cols land); the rest follow the xt remainder on SP.
        _mp_chunk(0, eng="scalar")
        _mp_chunk(3, eng="scalar")
        _xt_chunk(256, TOK)
        for mp in (1, 4, 2, 5):
            _mp_chunk(mp)
        # bo and Wo ride the SWDGE ring last: first needed by the out
        # projections near t~80us, so they must not delay the xt/wqkv
        # stream on the (globally serialized) DMA engines.
        bo_ap = bo[:]
        nc.gpsimd.dma_start(
            out=bo_sb,
            in_=bass.AP(tensor=bo_ap.tensor, offset=0, ap=[[0, P], [1, D]]))
        for k in range(KC):
            nc.gpsimd.dma_start(out=wo_sb[k], in_=wo[k * P:(k + 1) * P, :])
        nc.vector.memset(ones_row, 1.0)
        nc.vector.tensor_copy(out=bo_row, in_=bo_sb[0:1, :])
        # ones columns of v_aug (persistent; written once). On the vector
        # engine: the v_proj STT (also DVE) then needs no cross-engine wait.
        for t in range(TT):
            ones_view = vaug[t].rearrange("p (h x) -> p h x", x=VW)[:, :, DH:VW]
            nc.vector.memset(ones_view, 1.0)
        # Pre-observe the bias DMAs on the engines that consume them, so the
        # hot-loop ops carry only their PE wait.
        scratch = consts.tile([1, 4], F32, tag="scratch")
        nc.vector.tensor_copy(out=scratch[0:1, 0:1], in_=bv_sb[0:1, 0:1])
        nc.vector.tensor_copy(out=scratch[0:1, 1:2], in_=bo_sb[0:1, 0:1])
        nc.vector.tensor_copy(out=scratch[0:1, 2:3], in_=bqk_sb[0:1, 0:1])
        # dummy exp: pulls the ACT function-table load into the idle S1
        # phase instead of delaying the first real exp
        nc.scalar.activation(out=scratch[0:1, 3:4], in_=scratch[0:1, 0:1],
                             func=ACTF.Exp, scale=1.0)

        # ---- v-projection: one [128,1024] psum (768 used), 2 mm chains ---
        # Early v tiles alternate between the "sc" and (still idle) "ctx"
        # psum tags so the S1 phase isn't throttled by sc's 2-deep rotation.
        def v_proj(t, tag="sc"):
            ps = psum.tile([P, 1024], F32, tag=tag,
                           bufs=1 if tag == "ctx" else None)
            for k in range(KC):
                lhsT = xt_sb[k][:, t * P:(t + 1) * P]
                nc.tensor.matmul(ps[:, 0:512], lhsT,
                                 wqkv_sb[k][:, 2 * D:2 * D + 512],
                                 start=(k == 0), stop=(k == KC - 1))
                nc.tensor.matmul(ps[:, 512:768], lhsT,
                                 wqkv_sb[k][:, 2 * D + 512:3 * D],
                                 start=(k == 0), stop=(k == KC - 1))
            vview = vaug[t].rearrange("p (h x) -> p h x", x=VW)
            bview = bv_sb.rearrange("p (h x) -> p h x", x=DH)
            nc.vector.scalar_tensor_tensor(
                out=vview[:, 0:8, 0:DH],
                in0=ps[:, 0:512].rearrange("p (h x) -> p h x", x=DH),
                scalar=1.0, in1=bview[:, 0:8, :],
                op0=AOP.mult, op1=AOP.add)
            nc.vector.scalar_tensor_tensor(
                out=vview[:, 8:12, 0:DH],
                in0=ps[:, 512:768].rearrange("p (h x) -> p h x", x=DH),
                scalar=1.0, in1=bview[:, 8:12, :],
                op0=AOP.mult, op1=AOP.add)

        # ---- q/k projection unit: one m-tile, one token half -------------
        def qk_unit(m, tch, eng="vector"):
            ps = psum.tile([P, 512], F32, tag="qk")
            for k in range(KC):
                nc.tensor.matmul(
                    ps,
                    wqkv_sb[k][:, m * P:(m + 1) * P],
                    xt_sb[k][:, tch * 512:(tch + 1) * 512],
                    start=(k == 0), stop=(k == KC - 1))
            # PSUM->SBUF copy with per-partition bias add. Early units drain
            # on ACT (no exps yet); later ones on DVE (ACT is exp-saturated).
            if eng == "scalar":
                nc.scalar.activation(
                    out=qkT[m][:, tch * 512:(tch + 1) * 512], in_=ps,
                    func=ACTF.Identity, bias=bqk_sb[:, m:m + 1], scale=1.0)
            else:
                nc.vector.tensor_scalar(
                    out=qkT[m][:, tch * 512:(tch + 1) * 512], in0=ps,
                    scalar1=bqk_sb[:, m:m + 1], scalar2=None, op0=AOP.add)

        # ---- attention: scores+exp per (b, hp, kc); ctx per (b, hp) ------
        def scores(b, hp, kc, attn_tiles):
            ktile, qtile = qkT[HPAIRS + hp], qkT[hp]
            ps = psum.tile([P, 1024], F32, tag="sc")
            for hh in range(2):
                pr = slice(64 * hh, 64 * hh + 64)
                nc.tensor.matmul(
                    ps[:, hh * 512:(hh + 1) * 512],
                    ktile[pr, b * N + kc * P: b * N + (kc + 1) * P],
                    qtile[pr, b * N:(b + 1) * N],
                    start=True, stop=True)
            at = work.tile([P, 1024], BF16, tag="attn", bufs=14)
            nc.scalar.activation(out=at, in_=ps, func=ACTF.Exp,
                                 scale=1.0 / np.sqrt(DH))
            attn_tiles[kc] = at

        def ctx_group(b, hp, attn_tiles):
            # token-major ctx accumulation: slot s = (qc, hh) occupies cols
            # [128s, 128s+65): 64 v dims + softmax denominator (ones col).
            # PSUM zero-region semantics: start=True zeroes the full 2KB bank,
            # so exactly one start (first matmul into the bank) and one stop
            # (last matmul out of it); the other slots accumulate onto the
            # zeroed region.
            cps = psum.tile([P, 1024], F32, tag="ctx", bufs=1)
            for kc in range(4):
                at = attn_tiles[kc]
                for qc in range(4):
                    for hh in range(2):
                        s = qc * 2 + hh
                        h = 2 * hp + hh
                        nc.tensor.matmul(
                            cps[:, s * P: s * P + VW],
                            at[:, hh * 512 + qc * P: hh * 512 + (qc + 1) * P],
                            vaug[b * 4 + kc][:, h * VW:(h + 1) * VW],
                            start=(kc == 0 and s % 4 == 0),
                            stop=(kc == 3 and s % 4 == 3))
            # drain: reciprocal of the 8 denominators, normalize to bf16,
            # then XBAR-transpose each [128q, 128dh2] block into ctxT.
            bc = work.tile([P, 8], F32, tag="bc", bufs=6)
            den = cps.rearrange("p (s x) -> p s x", x=P)[:, :, DH:DH + 1]
            nc.vector.reciprocal(out=bc, in_=den)
            cns = [work.tile([P, P], BF16, tag="cn", bufs=12, name=f"cn{qc}")
                   for qc in range(4)]
            for s in range(8):
                qc, hh = s // 2, s % 2
                nc.vector.tensor_scalar(
                    out=cns[qc][:, hh * DH:(hh + 1) * DH],
                    in0=cps[:, s * P: s * P + DH],
                    scalar1=bc[:, s:s + 1], scalar2=None, op0=AOP.mult)
                if hh == 1:
                    nc.scalar.dma_start_transpose(
                        out=ctxT[b * HPAIRS + hp][:, qc * P:(qc + 1) * P],
                        in_=cns[qc])

        def out_proj_mm(b, tt_in_b):
            ps = psum.tile([P, 1024], F32, tag="sc")
            for hp in range(HPAIRS):
                lhsT = ctxT[b * HPAIRS + hp][:, tt_in_b * P:(tt_in_b + 1) * P]
                nc.tensor.matmul(ps[:, 0:512], lhsT, wo_sb[hp][:, 0:512],
                                 start=(hp == 0), stop=(hp == HPAIRS - 1))
                nc.tensor.matmul(ps[:, 512:768], lhsT, wo_sb[hp][:, 512:D],
                                 start=(hp == 0), stop=(hp == HPAIRS - 1))
            return ps

        def out_drain(b, tt_in_b, ps, chunks=1):
            t = b * 4 + tt_in_b
            o = work.tile([P, D], BF16, tag="out", bufs=6)
            edges = [round(D * i / chunks) for i in range(chunks + 1)]
            for c0, c1 in zip(edges, edges[1:]):
                nc.vector.scalar_tensor_tensor(
                    out=o[:, c0:c1], in0=ps[:, c0:c1], scalar=1.0,
                    in1=bo_sb[:, c0:c1], op0=AOP.mult, op1=AOP.add)
                nc.sync.dma_start(out=out[t * P:(t + 1) * P, c0:c1],
                                  in_=o[:, c0:c1])

        def out_proj(b, tt_in_b, chunks=1):
            out_drain(b, tt_in_b, out_proj_mm(b, tt_in_b), chunks)

        def out_proj_final(b, tt_in_b):
            # Last tile, three chains in separate psum tiles (PSUM dep
            # tracking is tile-granular) so earlier chunks' drains overlap
            # later chunks' matmuls. The last 128-col chunk accumulates its
            # bias via a K=1 ones-row matmul and drains through the idle
            # ACT engine and ACT DMA ring, fully parallel to the DVE/SP
            # tail of the earlier chunks.
            t = b * 4 + tt_in_b
            o = work.tile([P, D], BF16, tag="out", bufs=6)
            for c0, c1, ring in ((0, 256, nc.sync), (256, 512, nc.sync)):
                ps = psum.tile([P, 1024], F32, tag="sc", name=f"psf{c0}")
                for hp in range(HPAIRS):
                    lhsT = ctxT[b * HPAIRS + hp][:, tt_in_b * P:(tt_in_b + 1) * P]
                    nc.tensor.matmul(ps[:, 0:c1 - c0], lhsT, wo_sb[hp][:, c0:c1],
                                     start=(hp == 0), stop=(hp == HPAIRS - 1))
                nc.vector.scalar_tensor_tensor(
                    out=o[:, c0:c1], in0=ps[:, 0:c1 - c0], scalar=1.0,
                    in1=bo_sb[:, c0:c1], op0=AOP.mult, op1=AOP.add)
                ring.dma_start(out=out[t * P:(t + 1) * P, c0:c1],
                               in_=o[:, c0:c1])
            # last two 128-col chunks: bias via K=1 ones-row matmul, pure
            # ACT-copy drain and ACT-ring store — off the DVE/SP tail path
            for c0, c1 in ((512, 640), (640, D)):
                ps3 = psum.tile([P, 512], F32, tag="qk", name=f"psf{c0}")
                for hp in range(HPAIRS):
                    lhsT = ctxT[b * HPAIRS + hp][:, tt_in_b * P:(tt_in_b + 1) * P]
                    nc.tensor.matmul(ps3[:, 0:P], lhsT, wo_sb[hp][:, c0:c1],
                                     start=(hp == 0), stop=False)
                nc.tensor.matmul(ps3[:, 0:P], ones_row, bo_row[0:1, c0:c1],
                                 start=False, stop=True)
                nc.scalar.activation(out=o[:, c0:c1], in_=ps3[:, 0:P],
                                     func=ACTF.Identity, bias=0.0, scale=1.0)
                nc.scalar.dma_start(out=out[t * P:(t + 1) * P, c0:c1],
                                    in_=o[:, c0:c1])

        # ---- emission schedule (software pipeline) -----------------------
        # S1: batch-0 v tiles + the first qk units, overlapped with loads.
        for t in range(4):
            v_proj(t, tag="sc" if t % 2 == 0 else "ctx")
        qk_unit(0, 0)
        qk_unit(HPAIRS, 0)

        # S2: per head-pair, both batches' attention with qk/v/out units as
        # PE filler between the score matmuls (keeps PE busy while ACT exps).
        # Each iteration has 8 fill slots; the qk units for the NEXT pair go
        # in the batch-1 half so both halves stay PE-bound.
        for hp in range(HPAIRS):
            at0, at1 = {}, {}
            last = hp == HPAIRS - 1
            # filler slots F1..F8, None = skip
            F = [None] * 8
            F[0] = lambda hp=hp: qk_unit(hp, 1)
            F[1] = lambda hp=hp: qk_unit(HPAIRS + hp, 1)
            if hp == 0:
                F[0] = lambda: v_proj(4)
                F[1] = lambda: v_proj(5)
                F[2] = lambda: qk_unit(0, 1)
                F[3] = lambda: qk_unit(HPAIRS, 1)
                F[6] = lambda: v_proj(6)
                F[7] = lambda: v_proj(7)
            if not last:
                F[4] = lambda hp=hp: qk_unit(hp + 1, 0)
                F[5] = lambda hp=hp: qk_unit(HPAIRS + hp + 1, 0)

            def fill(i):
                if F[i] is not None:
                    F[i]()

            scores(0, hp, 0, at0)
            scores(0, hp, 1, at0)
            fill(0)
            scores(0, hp, 2, at0)
            fill(1)
            scores(0, hp, 3, at0)
            fill(2)
            fill(3)
            ctx_group(0, hp, at0)
            scores(1, hp, 0, at1)
            fill(4)
            scores(1, hp, 1, at1)
            fill(5)
            scores(1, hp, 2, at1)
            fill(6)
            # batch-0 ctxT completes in this iteration; out_proj(0,0)'s
            # matmuls are PE filler for the final pair's exps, but its
            # stores are deferred until after ctx_group(1,5)'s transposes
            # so the XBARs sit early in the HWDGE chain.
            ps00 = out_proj_mm(0, 0) if last else None
            scores(1, hp, 3, at1)
            fill(7)
            ctx_group(1, hp, at1)
            if last:
                out_drain(0, 0, ps00)

        # S3: remaining out projections; final tile drains in small chunks
        # so the PE->DVE->DMA tail is short.
        for tt_in_b in range(1, 4):
            out_proj(0, tt_in_b)
        for tt_in_b in range(3):
            out_proj(1, tt_in_b)
        out_proj_final(1, 3)


def _get_nc():
    global _CACHED_NC
    if _CACHED_NC is None:
        _CACHED_NC = _build_nc()
    return _CACHED_NC


def kernel(x, Wqkv, bqkv, Wo, bo):
    global LAST_EXEC_NS, LAST_RESULTS
    x = np.asarray(x, dtype=np.float32)
    wqkv_bf = np.asarray(Wqkv, dtype=np.float32).astype(BF16_NP)
    wo_bf = np.asarray(Wo, dtype=np.float32).astype(BF16_NP)
    bqkv_f = np.ascontiguousarray(np.asarray(bqkv, dtype=np.float32))
    bo_f = np.ascontiguousarray(np.asarray(bo, dtype=np.float32))

    in_maps = []
    for c in range(NCORES):
        xc = x[c * BPC:(c + 1) * BPC].reshape(TOK, D).T  # [768, 1024]
        in_maps.append({
            "xt": np.ascontiguousarray(xc).astype(BF16_NP),
            "wqkv": wqkv_bf,
            "bqkv": bqkv_f,
            "wo": wo_bf,
            "bo": bo_f,
        })

    nc = _get_nc()
    res = run_bass_kernel_spmd(nc, in_maps, list(range(NCORES)), trace=TRACE)
    LAST_EXEC_NS = res.exec_time_ns
    LAST_RESULTS = res
    outs = [np.asarray(res.results[c]["out"]).astype(np.float32)
            for c in range(NCORES)]
    return np.concatenate(outs, axis=0).reshape(B, N, D)
